# revision 5
# baseline (speedup 1.0000x reference)
"""Causal rotary self-attention Trainium2 kernel (8 NeuronCores).

Problem: B=4, N=1024, D=1024, H=16, DH=64.
  LayerNorm -> QKV proj -> RoPE(q,k) -> causal attention -> out proj.

Sharding: 8 cores = 4 batches x 2 head-halves (Megatron-style).  Each core
computes LN(x[b]) redundantly, projects its 8 heads' q/k/v with its weight
slice, runs attention for those heads, and produces a partial output
projection (bf16); the host sums the two partials per batch.

Per-core dataflow (all matmul inputs bf16, accumulation fp32):
  - LN on natural layout (tok partitions), fused (x-mu)*rstd via tensor_scalar;
    all 8 x tiles DMA up-front (x pool is fully buffered so DMA never waits
    on LN)
  - xn transposed 128x128 via TensorE into xnT (D partitions, tok free);
    PSUM evacuated by VectorE (bf16 2x mode)
  - q/k projected *transposed* (feature partitions, tok free): S^T scores need
    the dh contraction on partitions, so no post-RoPE transposes are needed
  - RoPE in transposed layout: rotate_half becomes +-32 partition-base-shifted
    tensor_tensor ops; the sin tensor is pre-shifted/signed on the host so both
    TT inputs share a partition base (HW verifier requirement)
  - S^T = kT.T @ qT per (head, k-block) over the causal region only
  - P^T = exp(S^T/8) on ScalarE straight out of PSUM; diagonal blocks masked
    with a triangular 0/1 tile; no max-subtraction (scores bounded, verified)
  - attn@V: lhsT = P^T block (already the needed layout), rhs = V with 64
    ones-columns appended -> rows 64..127 of the PSUM output all hold the
    softmax denominator (a free partition-broadcast); normalize is then a
    single reciprocal + tensor_tensor, no gpsimd broadcast needed
  - attn_out stays transposed (aoT); output projection consumes it as lhsT
  - emission interleaves the qk-projection + RoPE of head-pair m+1 (and the
    v-projection / output-projection chunks) under the exp-bound softmax of
    pair m so TensorE never drains while ScalarE works through the exps
"""

import numpy as np

B, N, D = 4, 1024, 1024
H, DH = 16, 64
EPS = 1e-5
P = 128
NHL = 8          # heads per core
FL = NHL * DH    # local features per core (512)

_cache = {}


def _build_module(has_beta, has_mask):
    import concourse.bass as bass
    import concourse.bacc as bacc
    import concourse.tile as tile
    import concourse.mybir as mybir
    from concourse.masks import make_identity

    f32 = mybir.dt.float32
    bf16 = mybir.dt.bfloat16
    AF = mybir.ActivationFunctionType
    OP = mybir.AluOpType

    nc = bacc.Bacc("TRN2", target_bir_lowering=False, debug=False, num_devices=8)

    x_in = nc.dram_tensor("x_in", [N, D], bf16, kind="ExternalInput").ap()
    wqk_in = nc.dram_tensor("wqk_in", [D, 2 * FL], bf16, kind="ExternalInput").ap()
    wv_in = nc.dram_tensor("wv_in", [D, FL], bf16, kind="ExternalInput").ap()
    wo_in = nc.dram_tensor("wo_in", [FL, D], bf16, kind="ExternalInput").ap()
    cos_in = nc.dram_tensor("cos_in", [P, N], bf16, kind="ExternalInput").ap()
    sinm_in = nc.dram_tensor("sinm_in", [P, N], bf16, kind="ExternalInput").ap()
    tri_in = nc.dram_tensor("tri_in", [P, P], bf16, kind="ExternalInput").ap()
    if has_beta:
        bqk_in = nc.dram_tensor("bqk_in", [P, 8], f32, kind="ExternalInput").ap()
        bv_in = nc.dram_tensor("bv_in", [1, FL], bf16, kind="ExternalInput").ap()
    if has_mask:
        madd_in = nc.dram_tensor("madd_in", [P, 8], f32, kind="ExternalInput").ap()
    out_p = nc.dram_tensor("out_p", [N, D], bf16, kind="ExternalOutput").ap()
    out_p2 = nc.dram_tensor("out_p2", [N, D], bf16, kind="ExternalOutput").ap()

    NT = N // P   # 8 token chunks
    ND = D // P   # 8 contraction chunks
    NIC = FL // P  # 4 inner chunks

    with tile.TileContext(nc) as tc:
        import contextlib
        ctx = contextlib.ExitStack()
        with ctx:
            consts = ctx.enter_context(tc.tile_pool(name="consts", bufs=1))
            big = ctx.enter_context(tc.tile_pool(name="big", bufs=1))
            x_pool = ctx.enter_context(tc.tile_pool(name="x_pool", bufs=8))
            xn_pool = ctx.enter_context(tc.tile_pool(name="xn_pool", bufs=8))
            stat = ctx.enter_context(tc.tile_pool(name="stat", bufs=4))
            qkt_pool = ctx.enter_context(tc.tile_pool(name="qkt_pool", bufs=3))
            tmp = ctx.enter_context(tc.tile_pool(name="tmp", bufs=3))
            pt_pool = ctx.enter_context(tc.tile_pool(name="pt_pool", bufs=4))
            small = ctx.enter_context(tc.tile_pool(name="small", bufs=2))
            out_pool = ctx.enter_context(tc.tile_pool(name="out_pool", bufs=3))
            psum = ctx.enter_context(tc.tile_pool(name="psum", bufs=2, space="PSUM"))

            # ---- x DMAs first (sync queue), all 8 tiles buffered ----
            x_tiles = []
            for ti in range(NT):
                x_t = x_pool.tile([P, D], bf16, name=f"x_t{ti}", tag="x")
                nc.sync.dma_start(out=x_t, in_=x_in[ti * P:(ti + 1) * P, :])
                x_tiles.append(x_t)

            # ---- weights / constants (gpsimd queue), in order of need ----
            wqk_sb = consts.tile([P, ND, 2 * FL], bf16)
            nc.gpsimd.dma_start(out=wqk_sb,
                              in_=wqk_in.rearrange("(dc p) f -> p dc f", p=P))
            cos_sb = consts.tile([P, N], bf16)
            nc.gpsimd.dma_start(out=cos_sb, in_=cos_in)
            sinm_sb = consts.tile([P, N], bf16)
            nc.gpsimd.dma_start(out=sinm_sb, in_=sinm_in)
            tri_sb = consts.tile([P, P], bf16)
            nc.gpsimd.dma_start(out=tri_sb, in_=tri_in)
            wv_sb = consts.tile([P, ND, FL], bf16)
            nc.gpsimd.dma_start(out=wv_sb,
                              in_=wv_in.rearrange("(dc p) f -> p dc f", p=P))
            if has_beta:
                bqk_sb = consts.tile([P, 8], f32)
                nc.gpsimd.dma_start(out=bqk_sb, in_=bqk_in)
                bv_sb = consts.tile([P, FL], bf16)
                nc.sync.dma_start(
                    out=bv_sb,
                    in_=type(bv_in)(tensor=bv_in.tensor, offset=0,
                                    ap=[[0, P]] + [list(p) for p in bv_in.ap[1:]]),
                )
            if has_mask:
                madd_sb = consts.tile([P, 8], f32)
                nc.gpsimd.dma_start(out=madd_sb, in_=madd_in)

            eps_t = consts.tile([P, 1], f32)
            nc.vector.memset(eps_t, EPS)
            ident = consts.tile([P, P], bf16)
            make_identity(nc, ident)

            # ---- LayerNorm (natural layout) ----
            xn_tiles = []
            for ti in range(NT):
                x_t = x_tiles[ti]
                st = stat.tile([P, 2, nc.vector.BN_STATS_DIM], f32,
                               name=f"st{ti}", tag="st")
                nc.vector.bn_stats(out=st[:, 0, :], in_=x_t[:, 0:512])
                nc.vector.bn_stats(out=st[:, 1, :], in_=x_t[:, 512:1024])
                mv = stat.tile([P, nc.vector.BN_AGGR_DIM], f32,
                               name=f"mv{ti}", tag="mv")
                nc.vector.bn_aggr(out=mv, in_=st)
                sd = stat.tile([P, 1], f32, name=f"sd{ti}", tag="sd")
                nc.scalar.activation(out=sd, in_=mv[:, 1:2], func=AF.Sqrt, bias=eps_t)
                rstd = stat.tile([P, 1], f32, name=f"rstd{ti}", tag="rstd")
                nc.vector.reciprocal(out=rstd, in_=sd)
                xn_t = xn_pool.tile([P, D], bf16, name=f"xn_t{ti}", tag="xn")
                nc.vector.tensor_scalar(
                    out=xn_t, in0=x_t, scalar1=mv[:, 0:1], scalar2=rstd,
                    op0=OP.subtract, op1=OP.mult)
                xn_tiles.append(xn_t)

            # ---- transpose xn -> xnT (D partitions, tok free), per token half --
            xnT_sb = big.tile([P, ND, N], bf16)
            for tg in (0, 4):
                for dc in range(ND):
                    tp = psum.tile([P, 512], bf16, tag="mm", name=f"tp{dc}_{tg}")
                    for j in range(4):
                        nc.tensor.transpose(
                            tp[:, j * P:(j + 1) * P],
                            xn_tiles[tg + j][:, dc * P:(dc + 1) * P], ident)
                    nc.vector.tensor_copy(
                        out=xnT_sb[:, dc, tg * P:tg * P + 512], in_=tp)

            # wo needed only for the output projection (last)
            wo_sb = consts.tile([P, NIC, D], bf16)
            nc.gpsimd.dma_start(out=wo_sb,
                              in_=wo_in.rearrange("(ic p) d -> p ic d", p=P))

            # ---- q/k projection (transposed out) with RoPE fused per fc ----
            rope_sb = big.tile([P, 2 * NIC, N], bf16)

            def rope_fc(fc, qkT):
                t1 = tmp.tile([P, N], bf16, name=f"t1_{fc}", tag="t1")
                t2 = tmp.tile([P, N], bf16, name=f"t2_{fc}", tag="t2")
                nc.vector.tensor_tensor(
                    out=t1, in0=qkT, in1=cos_sb, op=OP.mult)
                for (o, i) in ((0, 32), (32, 0), (64, 96), (96, 64)):
                    nc.vector.tensor_tensor(
                        out=t2[o:o + 32], in0=qkT[i:i + 32],
                        in1=sinm_sb[i:i + 32], op=OP.mult)
                nc.vector.tensor_tensor(
                    out=rope_sb[:, fc, :], in0=t1, in1=t2, op=OP.add)

            def qk_fc(fc):
                qkT = qkt_pool.tile([P, N], bf16, name=f"qkT{fc}", tag="qkT")
                for tc2 in range(2):
                    mm = psum.tile([P, 512], f32, tag="mm", name=f"qk{fc}_{tc2}")
                    for dc in range(ND):
                        nc.tensor.matmul(
                            mm, lhsT=wqk_sb[:, dc, fc * P:(fc + 1) * P],
                            rhs=xnT_sb[:, dc, tc2 * 512:(tc2 + 1) * 512],
                            start=(dc == 0), stop=(dc == ND - 1))
                    if has_beta:
                        nc.vector.tensor_scalar(
                            out=qkT[:, tc2 * 512:(tc2 + 1) * 512],
                            in0=mm, scalar1=bqk_sb[:, fc:fc + 1], scalar2=None,
                            op0=OP.add)
                    else:
                        nc.scalar.copy(
                            out=qkT[:, tc2 * 512:(tc2 + 1) * 512], in_=mm)
                rope_fc(fc, qkT)

            # ---- v projection (natural layout) + 64 ones columns ----
            v3 = big.tile([P, NT, NHL, P], bf16)
            nc.vector.memset(v3[:, :, :, DH:P], 1.0)

            def v_proj(kc):
                mm = psum.tile([P, 512], f32, tag="mm", name=f"v{kc}")
                for dc in range(ND):
                    nc.tensor.matmul(
                        mm, lhsT=xnT_sb[:, dc, kc * P:(kc + 1) * P],
                        rhs=wv_sb[:, dc, :],
                        start=(dc == 0), stop=(dc == ND - 1))
                if has_beta:
                    vtmp = tmp.tile([P, FL], bf16, name=f"vtmp{kc}", tag="vtmp")
                    nc.vector.tensor_add(
                        out=vtmp.rearrange("p (h c) -> p h c", h=NHL),
                        in0=mm.rearrange("p (h c) -> p h c", h=NHL),
                        in1=bv_sb.rearrange("p (h c) -> p h c", h=NHL))
                    nc.vector.tensor_copy(
                        out=v3[:, kc, :, 0:DH],
                        in_=vtmp.rearrange("p (h c) -> p h c", h=NHL))
                else:
                    nc.scalar.copy(
                        out=v3[:, kc, :, 0:DH],
                        in_=mm.rearrange("p (h c) -> p h c", h=NHL))

            # ---- attention ----
            aoT_sb = big.tile([P, NIC, N], bf16)
            all_pt = {}

            def phase1_pair(m):
                hs = (2 * m, 2 * m + 1)
                qTs = [rope_sb[(h % 2) * 64:(h % 2) * 64 + 64, h // 2, :]
                       for h in hs]
                kTs = [rope_sb[(h % 2) * 64:(h % 2) * 64 + 64, NIC + h // 2, :]
                       for h in hs]
                pts = [[], []]
                for ki in range(NT):
                    q0 = ki * P
                    span = N - q0
                    sps = []
                    for a, h in enumerate(hs):
                        pt = pt_pool.tile([P, span], bf16, tag=f"pt{ki}",
                                          name=f"pt{h}_{ki}")
                        sp = psum.tile([P, 1024], f32, tag=f"s{a}", bufs=1,
                                       name=f"s{h}_{ki}")
                        pts[a].append(pt)
                        sps.append(sp)
                    # alternate the two heads' score MMs so the PE runs them
                    # concurrently in disjoint row-group subarrays
                    for c in range((span + 511) // 512):
                        cw = min(512, span - c * 512)
                        for a in range(2):
                            nc.tensor.matmul(
                                sps[a][:, c * 512:c * 512 + cw],
                                lhsT=kTs[a][:, ki * P:(ki + 1) * P],
                                rhs=qTs[a][:, q0 + c * 512: q0 + c * 512 + cw],
                                start=True, stop=True)
                    for a, h in enumerate(hs):
                        if has_mask:
                            nc.scalar.activation(
                                out=pts[a][ki][:, 0:span], in_=sps[a][:, 0:span],
                                func=AF.Exp, scale=float(DH) ** -0.5,
                                bias=madd_sb[:, ki:ki + 1])
                        else:
                            nc.scalar.activation(
                                out=pts[a][ki][:, 0:span], in_=sps[a][:, 0:span],
                                func=AF.Exp, scale=float(DH) ** -0.5)
                        nc.vector.tensor_tensor(
                            out=pts[a][ki][:, 0:P], in0=pts[a][ki][:, 0:P],
                            in1=tri_sb, op=OP.mult)
                for a, h in enumerate(hs):
                    all_pt[h] = pts[a]

            def phase2(h):
                pt_tiles = all_pt.pop(h)
                pb2 = (h % 2) * 64
                ic = h // 2
                for cc in range(2):
                    clo, chi = cc * 512, (cc + 1) * 512
                    kis = [ki for ki in range(NT) if ki * P < chi]
                    av = psum.tile([P, 512], f32, tag="av", name=f"av{h}_{cc}")
                    for idx, ki in enumerate(kis):
                        qlo = max(clo, ki * P)
                        nc.tensor.matmul(
                            av[:, qlo - clo:512],
                            lhsT=v3[:, ki, h, :],
                            rhs=pt_tiles[ki][:, qlo - ki * P:chi - ki * P],
                            start=(idx == 0), stop=(idx == len(kis) - 1))
                    # rows DH..2*DH-1 of av hold the softmax denominator,
                    # already broadcast across 64 partitions by the ones block
                    den = small.tile([DH, 512], f32, name=f"den{h}_{cc}", tag="den")
                    nc.vector.tensor_copy(out=den, in_=av[DH:2 * DH, :])
                    rr = small.tile([DH, 512], f32, name=f"rr{h}_{cc}", tag="rr")
                    nc.vector.reciprocal_approx_fast(out=rr, in_=den)
                    nc.vector.tensor_tensor(
                        out=aoT_sb[pb2:pb2 + DH, ic, clo:chi],
                        in0=av[0:DH, :], in1=rr, op=OP.mult)

            def outproj_half(half, dst):
                for tci in range(NT):
                    mms = []
                    for n2 in range(2):
                        mm = psum.tile([P, 512], f32, tag="mm",
                                       name=f"op{half}_{tci}_{n2}")
                        mms.append(mm)
                    for ic in (2 * half, 2 * half + 1):
                        for n2 in range(2):
                            nc.tensor.matmul(
                                mms[n2], lhsT=aoT_sb[:, ic, tci * P:(tci + 1) * P],
                                rhs=wo_sb[:, ic, n2 * 512:(n2 + 1) * 512],
                                start=(ic == 2 * half), stop=(ic == 2 * half + 1))
                    for n2 in range(2):
                        ot = out_pool.tile([P, 512], bf16,
                                           name=f"ot{half}_{tci}_{n2}", tag="ot")
                        nc.scalar.copy(out=ot, in_=mms[n2])
                        nc.sync.dma_start(
                            out=dst[tci * P:(tci + 1) * P, n2 * 512:(n2 + 1) * 512],
                            in_=ot)

            # ---- software pipeline: projections of pair m+1 and v/out-proj
            # chunks fill TensorE while ScalarE works through pair m's exps ----
            qk_fc(0); qk_fc(4)
            phase1_pair(0)
            qk_fc(1); qk_fc(5)
            for kc in range(4):
                v_proj(kc)
            phase1_pair(1)
            for kc in range(4, NT):
                v_proj(kc)
            phase2(0); phase2(1)
            qk_fc(2); qk_fc(6)
            phase1_pair(2)
            phase2(2); phase2(3)
            outproj_half(0, out_p)
            qk_fc(3); qk_fc(7)
            phase1_pair(3)
            phase2(4); phase2(5)
            phase2(6); phase2(7)
            outproj_half(1, out_p2)

    nc.compile()
    return nc


def kernel(x, rotary_time_emb, x_mask, ln_gamma, ln_beta, w_qkv, w_out, b_out):
    import ml_dtypes
    from concourse import bass_utils

    bf = ml_dtypes.bfloat16
    x = np.asarray(x, np.float32)
    emb = np.asarray(rotary_time_emb, np.float32)
    x_mask = np.asarray(x_mask)
    ln_gamma = np.asarray(ln_gamma, np.float32)
    ln_beta = np.asarray(ln_beta, np.float32)
    w_qkv = np.asarray(w_qkv, np.float32)
    w_out = np.asarray(w_out, np.float32)
    b_out = np.asarray(b_out, np.float32)

    has_beta = bool(np.any(ln_beta != 0.0))
    has_mask = bool(np.any(~x_mask.astype(bool)))

    key = (has_beta, has_mask)
    if key not in _cache:
        _cache[key] = _build_module(has_beta, has_mask)
    nc = _cache[key]

    wg = w_qkv * ln_gamma[None, :]          # fold gamma into the projection
    inner = H * DH
    wq, wk, wv = wg[0:inner], wg[inner:2 * inner], wg[2 * inner:3 * inner]
    if has_beta:
        bias_qkv = w_qkv @ ln_beta          # per-feature bias from ln_beta
        bq, bk, bv = (bias_qkv[0:inner], bias_qkv[inner:2 * inner],
                      bias_qkv[2 * inner:3 * inner])

    cos = np.cos(emb)                       # (B, N, DH)
    sin = np.sin(emb)

    in_maps = []
    for core in range(8):
        b, hh = core // 2, core % 2
        sl = slice(hh * FL, (hh + 1) * FL)
        m = {
            "x_in": np.ascontiguousarray(x[b].astype(bf)),
            "wqk_in": np.ascontiguousarray(
                np.concatenate([wq[sl], wk[sl]], 0).T.astype(bf)),
            "wv_in": np.ascontiguousarray(wv[sl].T.astype(bf)),
            "wo_in": np.ascontiguousarray(w_out[:, sl].T.astype(bf)),
        }
        cT = cos[b].T                        # (DH, N)
        sT = sin[b].T
        cos2 = np.concatenate([cT, cT], 0)   # (128, N)
        sinm = np.concatenate([sT[32:64], -sT[0:32], sT[32:64], -sT[0:32]], 0)
        m["cos_in"] = np.ascontiguousarray(cos2.astype(bf))
        m["sinm_in"] = np.ascontiguousarray(sinm.astype(bf))
        k_idx = np.arange(P)[:, None]
        q_idx = np.arange(P)[None, :]
        m["tri_in"] = np.ascontiguousarray((k_idx <= q_idx).astype(bf))
        if has_beta:
            bqk = np.concatenate([bq[sl], bk[sl]], 0)      # (1024,)
            m["bqk_in"] = np.ascontiguousarray(
                bqk.reshape(8, P).T.astype(np.float32))    # [p, fc]
            m["bv_in"] = np.ascontiguousarray(bv[sl][None, :].astype(bf))
        if has_mask:
            madd = np.where(x_mask[b].astype(bool), 0.0, -30000.0)
            m["madd_in"] = np.ascontiguousarray(
                madd.reshape(8, P).T.astype(np.float32))   # [p, kc]
        in_maps.append(m)

    res = bass_utils.run_bass_kernel_spmd(nc, in_maps, core_ids=list(range(8)))

    out = np.empty((B, N, D), np.float32)
    for b in range(B):
        out[b] = (np.asarray(res.results[2 * b]["out_p"], np.float32)
                  + np.asarray(res.results[2 * b]["out_p2"], np.float32)
                  + np.asarray(res.results[2 * b + 1]["out_p"], np.float32)
                  + np.asarray(res.results[2 * b + 1]["out_p2"], np.float32))
    out += b_out[None, None, :]
    return out
